# revision 19
# baseline (speedup 1.0000x reference)
"""Multi-head causal attention (b=2, s=2048, d=1024, h=16) on 8 TRN2 cores.

Sharding: batch (2) x head-groups (4 heads each) -> 8 cores, Megatron-style.
Each core: QKV col-sliced projections (d -> 256), causal attention for its 4
heads, row-sliced output projection producing a partial [2048, 1024] output.
Host sums the 4 partials per batch and adds the output bias.

Device kernel layout:
  - x arrives pre-transposed (xT [1024, 2048]) and is streamed ONCE; q, k and
    v are all projected from the same resident [128, 8*512] x block per
    512-query chunk.
  - q, k are projected UNPADDED: head pair g = h//2 shares a psum tile
    ([128, 512]: head 2g on partitions 0-63, head 2g+1 on 64-127), halving
    the projection matmul count vs padded weights.
  - scores still contract over the FULL 128 partitions (half-array matmuls
    do not register as PE activity for the HAM clock gate, which then holds
    the whole attention phase at 1.2 GHz — measured): kT is stored per-head
    with the head's 64 rows at their natural partition range and the other
    64 rows zeroed once at startup; qT stays pair-packed and streams as the
    moving operand, its other-head rows multiplying the zero kT rows.
  - v natural ([s, head_dim]) with an extra ones column per head so the
    softmax denominator falls out of the ctx matmul (row 64 of ctx PSUM).
  - scores are computed transposed (p^T[j, i]) so the ctx matmul needs no
    transposes anywhere; softmax uses no max-subtraction (scores are O(5)
    for this distribution; exp is safe in fp32).
  - the Scalar (ACT) engine runs ONLY the exp stream — it is the attention
    bottleneck, so projection of x-block sc+1 is INJECTED into the attention
    pass stream of query block sc (like the out-proj injection) to overlap
    the whole projection phase with exp instead of serializing phases.
  - all matmuls run as float32r (fp22 multiply), full PE speed at
    moving >= 256 on TRN2.
"""

import ml_dtypes
import numpy as np

import concourse.bass as bass
import concourse.tile as tile
from concourse import bacc
from concourse import mybir
from concourse import bass_utils

F32 = mybir.dt.float32
F32R = mybir.dt.float32r
BF16 = mybir.dt.bfloat16
EXP = mybir.ActivationFunctionType.Exp

B, S, D, H = 2, 2048, 1024, 16
HG = 4                  # heads per core
E = 64                  # head dim
DG = HG * E             # 256, d-slice per core
NC = 8                  # cores
IT = 512                # query tile (moving dim of both attention matmuls)
JT = 128                # key tile
KC = D // 128           # 8 contraction chunks for projections
NSC = S // IT           # 4 s-chunks of 512
NST = S // JT           # 16 s-tiles of 128
SCALE = 1.0 / np.sqrt(E)

_CACHE = {}


def _build():
    nc = bacc.Bacc("TRN2", target_bir_lowering=False, debug=False)

    xT = nc.dram_tensor("xT", [D, S], BF16, kind="ExternalInput").ap()
    wq = nc.dram_tensor("wq", [D, DG], BF16, kind="ExternalInput").ap()
    wk = nc.dram_tensor("wk", [D, DG], BF16, kind="ExternalInput").ap()
    wv = nc.dram_tensor("wv", [D, DG], BF16, kind="ExternalInput").ap()
    wo = nc.dram_tensor("wo", [DG, D], F32R, kind="ExternalInput").ap()
    tri = nc.dram_tensor("tri", [JT, JT], F32, kind="ExternalInput").ap()
    out = nc.dram_tensor("out", [S, D], BF16, kind="ExternalOutput").ap()

    with tile.TileContext(nc) as tc:
        from contextlib import ExitStack

        with ExitStack() as ctx:
            pers = ctx.enter_context(tc.tile_pool(name="pers", bufs=1))

            wq_sb = pers.tile([128, KC * DG], BF16, tag="wq")         # 4 KB/p
            wk_sb = pers.tile([128, KC * DG], BF16, tag="wk")
            wv_sb = pers.tile([128, KC * DG], BF16, tag="wv")
            wo_sb = pers.tile([128, 2 * D], F32R, tag="wo")           # 8 KB/p
            tri_sb = pers.tile([JT, JT], F32, tag="tri")
            qT_sb = pers.tile([128, 2 * S], BF16, tag="qT")           # 8 KB/p
            kT_sb = pers.tile([128, HG * S], BF16, tag="kT")          # 16 KB/p
            v_sb = pers.tile([128, NST * (HG * (E + 1))], BF16, tag="v")  # 8.1 KB/p
            cx_sb = pers.tile([128, 2 * S], F32R, tag="cx")           # 16 KB/p
            ones_sb = pers.tile([128, 128], F32R, tag="ones")
            dn_a = pers.tile([128, IT], F32R, tag="dnpa")
            dn_b = pers.tile([128, IT], F32R, tag="dnpb")

            VW = HG * (E + 1)  # 260, v-row width per s-tile

            # ---- constants via engine memsets (a DMA'd ones-column scatter
            # measured 6+ us of descriptor grind). memset can't lower f32r
            # dsts — bitcast to plain f32 first. ----
            v3 = v_sb.rearrange("p (g x) -> p g x", x=E + 1)
            nc.vector.memset(ones_sb.bitcast(F32)[:], 1.0)
            nc.vector.memset(v3[:, :, E:E + 1], 1.0)
            nc.vector.memset(dn_a.bitcast(F32)[:], 0.0)
            nc.vector.memset(dn_b.bitcast(F32)[:], 0.0)
            # zero the dead 64-row half of each kT head block once; scores
            # matmuls then contract over the full 128 partitions
            for h in range(HG):
                po = 64 * (h % 2)
                dead = slice(64 - po, 128 - po)
                eng = nc.gpsimd if h % 2 == 0 else nc.vector
                eng.memset(kT_sb[dead, h * S:(h + 1) * S], 0.0)

            # ---- input loads. DMA issue is ~0.8 us apiece on an engine
            # queue, so loads are spread across idle engine queues (sync
            # keeps the critical wq/x chain, scalar takes wk/wo, gpsimd
            # takes wv behind its memsets), ordered so the first projection
            # matmul's chunks land first. ----
            wq3 = wq.rearrange("(k p) c -> p k c", p=128)
            wk3 = wk.rearrange("(k p) c -> p k c", p=128)
            wv3 = wv.rearrange("(k p) c -> p k c", p=128)
            wqs3 = wq_sb.rearrange("p (k c) -> p k c", c=DG)
            wks3 = wk_sb.rearrange("p (k c) -> p k c", c=DG)
            wvs3 = wv_sb.rearrange("p (k c) -> p k c", c=DG)
            h0, h1 = slice(0, 4), slice(4, 8)
            nc.sync.dma_start(tri_sb[:], tri[:])
            nc.sync.dma_start(wqs3[:, h0, :], wq3[:, h0, :])
            for hs in (h0, h1):
                nc.gpsimd.dma_start(wvs3[:, hs, :], wv3[:, hs, :])

            # warm the HAM clock gate while inputs stream: ~4.3 us of
            # dummy matmuls releases the 4/8 clock throttle right when the
            # first projection chunks land (result never read)
            with tc.tile_pool(name="wup", bufs=1, space="PSUM") as wup:
                wps = wup.tile([128, 128], F32, name="wps", tag="w")
                for i in range(72):
                    nc.tensor.matmul(wps[:], lhsT=ones_sb[:], rhs=ones_sb[:],
                                     start=(i == 0), stop=(i == 71))

            xv = xT.rearrange("(k p) s -> p k s", p=128)
            xp = ctx.enter_context(tc.tile_pool(name="xp", bufs=2))
            xts = []
            for sc in range(NSC):
                xt = xp.tile([128, KC * IT], BF16, tag="xt", name="xt")
                xt3 = xt.rearrange("p (k j) -> p k j", j=IT)
                if sc == 0:
                    nc.sync.dma_start(xt3[:, h0, :], xv[:, h0, 0:IT])
                    nc.scalar.dma_start(xt3[:, h1, :], xv[:, h1, 0:IT])
                    nc.scalar.dma_start(wqs3[:, h1, :], wq3[:, h1, :])
                    for hs in (h0, h1):
                        nc.scalar.dma_start(wks3[:, hs, :], wk3[:, hs, :])
                    for p in range(2):
                        nc.scalar.dma_start(
                            wo_sb[:, p * D:(p + 1) * D], wo[p * 128:(p + 1) * 128, :])
                else:
                    nc.sync.dma_start(xt3[:, :, :], xv[:, :, sc * IT:(sc + 1) * IT])
                xts.append(xt)

            # ---- projection emitters for one x block (q/k -> pair-packed
            # qT / zero-padded kT, v -> natural + ones col). Returned as a
            # flat list of single-matmul emitters so they can be injected
            # into the attention pass stream. ----
            def proj_emitters(sc, pool):
                ems = []
                xt = xts[sc]

                def qk_step(k, w_sb, g, hold, is_q):
                    def em():
                        if "ps" not in hold:
                            hold["ps"] = pool.tile([128, IT], F32, name="ps", tag="ps")
                        ps = hold["ps"]
                        nc.tensor.matmul(
                            ps[:],
                            lhsT=w_sb[:, k * DG + g * 128: k * DG + (g + 1) * 128],
                            rhs=xt[:, k * IT:(k + 1) * IT],
                            start=(k == 0), stop=(k == KC - 1),
                        )
                        if k == KC - 1:
                            if is_q:
                                nc.vector.tensor_copy(
                                    qT_sb[:, g * S + sc * IT: g * S + (sc + 1) * IT], ps[:])
                            else:
                                for hp in range(2):
                                    h = 2 * g + hp
                                    rows = slice(64 * hp, 64 * hp + 64)
                                    nc.vector.tensor_copy(
                                        kT_sb[rows, h * S + sc * IT: h * S + (sc + 1) * IT],
                                        ps[rows, :])
                    return em

                def v_step(k, st, hold):
                    def em():
                        if "ps" not in hold:
                            hold["ps"] = pool.tile([128, IT], F32, name="psv", tag="ps")
                        ps = hold["ps"]
                        nc.tensor.matmul(
                            ps[:, 0:DG],
                            lhsT=xt[:, k * IT + st * JT: k * IT + st * JT + JT],
                            rhs=wv_sb[:, k * DG:(k + 1) * DG],
                            start=(k == 0), stop=(k == KC - 1),
                        )
                        if k == KC - 1:
                            gst = sc * 4 + st
                            dst3 = v_sb[:, gst * VW:(gst + 1) * VW].rearrange(
                                "p (g x) -> p g x", x=E + 1)
                            nc.vector.tensor_copy(
                                dst3[:, :, 0:E],
                                ps[:, 0:DG].rearrange("p (g x) -> p g x", x=E))
                    return em

                for w_sb, is_q in ((wq_sb, True), (wk_sb, False)):
                    for g in range(2):
                        hold = {}
                        for k in range(KC):
                            ems.append(qk_step(k, w_sb, g, hold, is_q))
                for st in range(4):
                    hold = {}
                    for k in range(KC):
                        ems.append(v_step(k, st, hold))
                return ems

            # ---- dense prologue: project block 0 (own 3-buf psum scope) ----
            with tc.tile_pool(name="prop", bufs=3, space="PSUM") as prop:
                for em in proj_emitters(0, prop):
                    em()

            # ---- attention + injected projection / output projection ----
            # Flat software pipeline: ctx matmuls are emitted SKEW att-passes
            # after their scores matmul so PE never stalls on ACT's exp; the
            # normalize chain is staged in even later; out-proj for query
            # block ti and the projection of x block ti+1 are injected into
            # the attention stream of block ti.
            SK = 2
            INJ = {0: 5, 1: 3, 2: 2, 3: 0}   # proj emitters popped per pass
            with tc.tile_pool(name="prp", bufs=1, space="PSUM") as prp, \
                 tc.tile_pool(name="scp", bufs=3, space="PSUM") as scp, \
                 tc.tile_pool(name="cxp", bufs=2, space="PSUM") as cxp, \
                 tc.tile_pool(name="opp", bufs=2, space="PSUM") as opp, \
                 tc.tile_pool(name="pp_sb", bufs=6) as p_pool, \
                 tc.tile_pool(name="rb", bufs=2) as rbp, \
                 tc.tile_pool(name="ot", bufs=2) as otp:

                ctx_q = []    # (emit_fn, end_of_group_fn | None)
                due_q = []    # (passes_left, emit_fn) for staged normalize
                op_q = []     # pending out-proj emitters from previous block
                proj_q = []   # pending projection emitters for block ti+1
                norms_open = [0]  # groups whose cx write is not yet emitted
                inj_rate = [0]

                def emit_op(ti):
                    for it_ in range(4 * ti, 4 * ti + 4):
                        hold = {}
                        for dc in range(2):
                            def go(it_=it_, dc=dc, hold=hold):
                                ps = opp.tile([128, IT], F32, tag="ops")
                                for pair in range(2):
                                    nc.tensor.matmul(
                                        ps[:],
                                        lhsT=cx_sb[:, pair * S + it_ * JT: pair * S + it_ * JT + JT],
                                        rhs=wo_sb[:, pair * D + dc * IT: pair * D + (dc + 1) * IT],
                                        start=(pair == 0), stop=(pair == 1),
                                    )
                                ot = otp.tile([128, IT], BF16, name="ot", tag="ott")
                                nc.vector.tensor_copy(ot[:], ps[:])
                                eng = nc.sync
                                if it_ >= 12:
                                    eng = (nc.sync, nc.scalar, nc.gpsimd)[((it_ - 12) * 2 + dc) % 3]
                                eng.dma_start(
                                    out[it_ * JT:(it_ + 1) * JT, dc * IT:(dc + 1) * IT], ot[:])
                            op_q.append(go)

                norm_count = [0]

                def norm_stage_a(cps):
                    dn = dn_a if norm_count[0] % 2 == 0 else dn_b
                    norm_count[0] += 1
                    nc.vector.tensor_copy(dn[0:1, :], cps[E:E + 1, :])
                    return dn

                def norm_stage_b(cps, dn, h, ti):
                    qb, po = h // 2, 64 * (h % 2)
                    dnb = opp.tile([128, IT], F32, name="dnb", tag="ops")
                    nc.tensor.matmul(dnb[:], lhsT=ones_sb[:], rhs=dn[:],
                                     start=True, stop=True)
                    rcp = rbp.tile([128, IT], F32, tag="rcp")
                    nc.vector.reciprocal_approx_fast(rcp[0:E, :], dnb[0:E, :])
                    nc.vector.tensor_mul(
                        cx_sb[po:po + E, qb * S + ti * IT: qb * S + (ti + 1) * IT],
                        cps[0:E, :], rcp[0:E, :],
                    )
                    norms_open[0] -= 1

                def tick():
                    """Advance the pipeline by one att pass."""
                    for _ in range(inj_rate[0]):
                        if proj_q:
                            proj_q.pop(0)()
                    for e in list(due_q):
                        e[0] -= 1
                        if e[0] <= 0:
                            e[1]()
                            due_q.remove(e)
                    # out-proj reads cx, so it may only be emitted once the
                    # normalize stages that write cx have all been emitted
                    if op_q and not due_q and norms_open[0] == 0:
                        op_q.pop(0)()

                def drain_ctx():
                    emit, group_end = ctx_q.pop(0)
                    emit()
                    if group_end is not None:
                        group_end()

                for ti in range(NSC):
                    # block ti's projection must be fully emitted before its
                    # scores go on the PE queue (same-queue deadlock else)
                    while proj_q:
                        proj_q.pop(0)()
                    if ti + 1 < NSC:
                        proj_q.extend(proj_emitters(ti + 1, prp))
                    inj_rate[0] = INJ[ti]
                    njt = (IT // JT) * ti + (IT // JT)
                    for h in range(HG):
                        g = h // 2
                        cps = cxp.tile([128, IT], F32, tag="cps")
                        for jj in range(njt):
                            d = jj - (IT // JT) * ti
                            o = max(d, 0) * JT        # first valid query column
                            sp = scp.tile([128, IT], F32, tag="sp")
                            nc.tensor.matmul(
                                sp[:, o:IT],
                                lhsT=kT_sb[:, h * S + jj * JT: h * S + jj * JT + JT],
                                rhs=qT_sb[:, g * S + ti * IT + o: g * S + (ti + 1) * IT],
                                start=True, stop=True,
                            )
                            pt = p_pool.tile([128, IT], BF16, tag="pt")
                            nc.scalar.activation(pt[:, o:IT], sp[:, o:IT], EXP, scale=SCALE)
                            if d >= 0:
                                nc.gpsimd.tensor_mul(pt[:, o:o + JT], pt[:, o:o + JT], tri_sb[:])

                            def emit_ctx(cps=cps, pt=pt, h=h, jj=jj, o=o, njt=njt):
                                nc.tensor.matmul(
                                    cps[0:E + 1, o:IT],
                                    lhsT=v_sb[:, jj * VW + h * (E + 1): jj * VW + (h + 1) * (E + 1)],
                                    rhs=pt[:, o:IT],
                                    start=(jj == 0), stop=(jj == njt - 1),
                                )
                            group_end = None
                            if jj == njt - 1:
                                norms_open[0] += 1
                                def group_end(cps=cps, h=h, ti=ti):
                                    def stage_a(cps=cps, h=h, ti=ti):
                                        dn = norm_stage_a(cps)
                                        def stage_b(cps=cps, dn=dn, h=h, ti=ti):
                                            norm_stage_b(cps, dn, h, ti)
                                            due_q.append([2, lambda: None])
                                        due_q.append([4, stage_b])
                                    due_q.append([1, stage_a])
                            ctx_q.append((emit_ctx, group_end))
                            if len(ctx_q) > SK:
                                drain_ctx()
                            tick()
                    emit_op(ti)

                while ctx_q:
                    drain_ctx()
                    tick()
                for _ in range(60):
                    if not due_q and not op_q and not proj_q:
                        break
                    tick()
                assert not due_q and not op_q and not proj_q
                assert norms_open[0] == 0

    nc.compile()
    return nc


def _tri():
    # tri[jp, ic] = 1 where ic >= jp (keep), 0 above the causal boundary
    i = np.arange(JT)
    return (i[None, :] >= i[:, None]).astype(np.float32)


def _in_maps(x, Wq, Wk, Wv, Wo):
    tri = _tri()
    maps = []
    for c in range(NC):
        b, g = c // (NC // B), c % (NC // B)
        maps.append({
            "xT": np.ascontiguousarray(x[b].T).astype(ml_dtypes.bfloat16),
            "wq": np.ascontiguousarray(Wq[:, g * DG:(g + 1) * DG]).astype(ml_dtypes.bfloat16),
            "wk": np.ascontiguousarray(Wk[:, g * DG:(g + 1) * DG]).astype(ml_dtypes.bfloat16),
            "wv": np.ascontiguousarray(Wv[:, g * DG:(g + 1) * DG]).astype(ml_dtypes.bfloat16),
            "wo": np.ascontiguousarray(Wo[g * DG:(g + 1) * DG, :]),
            "tri": tri,
        })
    return maps


def run(x, Wq, Wk, Wv, Wo, bo, trace=False):
    if "nc" not in _CACHE:
        _CACHE["nc"] = _build()
    nc = _CACHE["nc"]
    res = bass_utils.run_bass_kernel_spmd(
        nc, _in_maps(x, Wq, Wk, Wv, Wo), core_ids=list(range(NC)), trace=trace,
    )
    parts = [np.asarray(res.results[c]["out"]).astype(np.float32) for c in range(NC)]
    gpb = NC // B
    full = np.stack([sum(parts[b * gpb + 1: (b + 1) * gpb], parts[b * gpb]) for b in range(B)])
    full = full + np.asarray(bo, np.float32)[None, None, :]
    return full.astype(np.float32), res


def kernel(x, Wq, Wk, Wv, Wo, bo):
    x = np.asarray(x, np.float32)
    full, _ = run(x, np.asarray(Wq, np.float32), np.asarray(Wk, np.float32),
                  np.asarray(Wv, np.float32), np.asarray(Wo, np.float32),
                  np.asarray(bo, np.float32))
    return full


# revision 20
# speedup vs baseline: 1.2404x; 1.2404x over previous
"""Multi-head causal attention (b=2, s=2048, d=1024, h=16) on 8 TRN2 cores.

Sharding: batch (2) x head-groups (4 heads each) -> 8 cores, Megatron-style.
Each core: QKV col-sliced projections (d -> 256), causal attention for its 4
heads, row-sliced output projection producing a partial [2048, 1024] output.
Host sums the 4 partials per batch and adds the output bias.

Device kernel layout:
  - x arrives pre-transposed (xT [1024, 2048]) and is streamed ONCE; q, k and
    v are all projected from the same resident [128, 8*512] x block per
    512-query chunk.
  - q, k are projected UNPADDED: head pair g = h//2 shares a psum tile
    ([128, 512]: head 2g on partitions 0-63, head 2g+1 on 64-127), halving
    the projection matmul count vs padded weights.
  - scores still contract over the FULL 128 partitions (half-array matmuls
    do not register as PE activity for the HAM clock gate, which then holds
    the whole attention phase at 1.2 GHz — measured): kT is stored per-head
    with the head's 64 rows at their natural partition range and the other
    64 rows zeroed once at startup; qT stays pair-packed and streams as the
    moving operand, its other-head rows multiplying the zero kT rows.
  - v natural ([s, head_dim]) with an extra ones column per head so the
    softmax denominator falls out of the ctx matmul (row 64 of ctx PSUM).
  - scores are computed transposed (p^T[j, i]) so the ctx matmul needs no
    transposes anywhere; softmax uses no max-subtraction (scores are O(5)
    for this distribution; exp is safe in fp32).
  - the Scalar (ACT) engine runs ONLY the exp stream — it is the attention
    bottleneck, so projection of x-block sc+1 is INJECTED into the attention
    pass stream of query block sc (like the out-proj injection) to overlap
    the whole projection phase with exp instead of serializing phases.
  - all matmuls run as float32r (fp22 multiply), full PE speed at
    moving >= 256 on TRN2.
"""

import ml_dtypes
import numpy as np

import concourse.bass as bass
import concourse.tile as tile
from concourse import bacc
from concourse import mybir
from concourse import bass_utils

F32 = mybir.dt.float32
F32R = mybir.dt.float32r
BF16 = mybir.dt.bfloat16
EXP = mybir.ActivationFunctionType.Exp

B, S, D, H = 2, 2048, 1024, 16
HG = 4                  # heads per core
E = 64                  # head dim
DG = HG * E             # 256, d-slice per core
NC = 8                  # cores
IT = 512                # query tile (moving dim of both attention matmuls)
JT = 128                # key tile
KC = D // 128           # 8 contraction chunks for projections
NSC = S // IT           # 4 s-chunks of 512
NST = S // JT           # 16 s-tiles of 128
SCALE = 1.0 / np.sqrt(E)

_CACHE = {}


def _build():
    nc = bacc.Bacc("TRN2", target_bir_lowering=False, debug=False)

    xT = nc.dram_tensor("xT", [D, S], BF16, kind="ExternalInput").ap()
    wq = nc.dram_tensor("wq", [D, DG], BF16, kind="ExternalInput").ap()
    wk = nc.dram_tensor("wk", [D, DG], BF16, kind="ExternalInput").ap()
    wv = nc.dram_tensor("wv", [D, DG], BF16, kind="ExternalInput").ap()
    wo = nc.dram_tensor("wo", [DG, D], F32R, kind="ExternalInput").ap()
    tri = nc.dram_tensor("tri", [JT, JT], F32, kind="ExternalInput").ap()
    out = nc.dram_tensor("out", [S, D], BF16, kind="ExternalOutput").ap()

    with tile.TileContext(nc) as tc:
        from contextlib import ExitStack

        with ExitStack() as ctx:
            pers = ctx.enter_context(tc.tile_pool(name="pers", bufs=1))

            wq_sb = pers.tile([128, KC * DG], BF16, tag="wq")         # 4 KB/p
            wk_sb = pers.tile([128, KC * DG], BF16, tag="wk")
            wv_sb = pers.tile([128, KC * DG], BF16, tag="wv")
            wo_sb = pers.tile([128, 2 * D], F32R, tag="wo")           # 8 KB/p
            tri_sb = pers.tile([JT, JT], F32, tag="tri")
            qT_sb = pers.tile([128, 2 * S], BF16, tag="qT")           # 8 KB/p
            kT_sb = pers.tile([128, HG * S], BF16, tag="kT")          # 16 KB/p
            v_sb = pers.tile([128, NST * (HG * (E + 1))], BF16, tag="v")  # 8.1 KB/p
            cx_sb = pers.tile([128, 2 * S], F32R, tag="cx")           # 16 KB/p
            ones_sb = pers.tile([128, 128], F32R, tag="ones")
            dn_a = pers.tile([128, IT], F32R, tag="dnpa")
            dn_b = pers.tile([128, IT], F32R, tag="dnpb")

            VW = HG * (E + 1)  # 260, v-row width per s-tile

            # ---- constants via engine memsets (a DMA'd ones-column scatter
            # measured 6+ us of descriptor grind). memset can't lower f32r
            # dsts — bitcast to plain f32 first. ----
            v3 = v_sb.rearrange("p (g x) -> p g x", x=E + 1)
            nc.vector.memset(ones_sb.bitcast(F32)[:], 1.0)
            nc.vector.memset(v3[:, :, E:E + 1], 1.0)
            nc.vector.memset(dn_a.bitcast(F32)[:], 0.0)
            nc.vector.memset(dn_b.bitcast(F32)[:], 0.0)
            # zero the dead 64-row half of each kT head block once; scores
            # matmuls then contract over the full 128 partitions
            for h in range(HG):
                po = 64 * (h % 2)
                dead = slice(64 - po, 128 - po)
                eng = nc.gpsimd if h % 2 == 0 else nc.vector
                eng.memset(kT_sb[dead, h * S:(h + 1) * S], 0.0)

            # ---- input loads. DMA issue is ~0.8 us apiece on an engine
            # queue, so loads are spread across idle engine queues (sync
            # keeps the critical wq/x chain, scalar takes wk/wo, gpsimd
            # takes wv behind its memsets), ordered so the first projection
            # matmul's chunks land first. ----
            wq3 = wq.rearrange("(k p) c -> p k c", p=128)
            wk3 = wk.rearrange("(k p) c -> p k c", p=128)
            wv3 = wv.rearrange("(k p) c -> p k c", p=128)
            wqs3 = wq_sb.rearrange("p (k c) -> p k c", c=DG)
            wks3 = wk_sb.rearrange("p (k c) -> p k c", c=DG)
            wvs3 = wv_sb.rearrange("p (k c) -> p k c", c=DG)
            h0, h1 = slice(0, 4), slice(4, 8)
            nc.sync.dma_start(tri_sb[:], tri[:])
            nc.sync.dma_start(wqs3[:, h0, :], wq3[:, h0, :])
            for hs in (h0, h1):
                nc.gpsimd.dma_start(wvs3[:, hs, :], wv3[:, hs, :])

            # warm the HAM clock gate while inputs stream: ~4.3 us of
            # dummy matmuls releases the 4/8 clock throttle right when the
            # first projection chunks land (result never read)
            with tc.tile_pool(name="wup", bufs=1, space="PSUM") as wup:
                wps = wup.tile([128, 128], F32, name="wps", tag="w")
                for i in range(40):
                    nc.tensor.matmul(wps[:], lhsT=ones_sb[:], rhs=ones_sb[:],
                                     start=(i == 0), stop=(i == 39))

            xv = xT.rearrange("(k p) s -> p k s", p=128)
            xp = ctx.enter_context(tc.tile_pool(name="xp", bufs=2))
            xts = []
            for sc in range(NSC):
                xt = xp.tile([128, KC * IT], BF16, tag="xt", name="xt")
                xt3 = xt.rearrange("p (k j) -> p k j", j=IT)
                if sc == 0:
                    nc.sync.dma_start(xt3[:, h0, :], xv[:, h0, 0:IT])
                    nc.scalar.dma_start(xt3[:, h1, :], xv[:, h1, 0:IT])
                    nc.scalar.dma_start(wqs3[:, h1, :], wq3[:, h1, :])
                    for hs in (h0, h1):
                        nc.scalar.dma_start(wks3[:, hs, :], wk3[:, hs, :])
                    for p in range(2):
                        nc.scalar.dma_start(
                            wo_sb[:, p * D:(p + 1) * D], wo[p * 128:(p + 1) * 128, :])
                else:
                    nc.sync.dma_start(xt3[:, :, :], xv[:, :, sc * IT:(sc + 1) * IT])
                xts.append(xt)

            # ---- projection emitters for one x block (q/k -> pair-packed
            # qT / zero-padded kT, v -> natural + ones col). Returned as a
            # flat list of single-matmul emitters so they can be injected
            # into the attention pass stream. ----
            def proj_emitters(sc, pool):
                ems = []
                xt = xts[sc]

                def qk_step(k, w_sb, g, hold, is_q):
                    def em():
                        if "ps" not in hold:
                            hold["ps"] = pool.tile([128, IT], F32, name="ps", tag="ps")
                        ps = hold["ps"]
                        nc.tensor.matmul(
                            ps[:],
                            lhsT=w_sb[:, k * DG + g * 128: k * DG + (g + 1) * 128],
                            rhs=xt[:, k * IT:(k + 1) * IT],
                            start=(k == 0), stop=(k == KC - 1),
                        )
                        if k == KC - 1:
                            if is_q:
                                nc.vector.tensor_copy(
                                    qT_sb[:, g * S + sc * IT: g * S + (sc + 1) * IT], ps[:])
                            else:
                                for hp in range(2):
                                    h = 2 * g + hp
                                    rows = slice(64 * hp, 64 * hp + 64)
                                    nc.vector.tensor_copy(
                                        kT_sb[rows, h * S + sc * IT: h * S + (sc + 1) * IT],
                                        ps[rows, :])
                    return em

                def v_step(k, st, hold):
                    def em():
                        if "ps" not in hold:
                            hold["ps"] = pool.tile([128, IT], F32, name="psv", tag="ps")
                        ps = hold["ps"]
                        nc.tensor.matmul(
                            ps[:, 0:DG],
                            lhsT=xt[:, k * IT + st * JT: k * IT + st * JT + JT],
                            rhs=wv_sb[:, k * DG:(k + 1) * DG],
                            start=(k == 0), stop=(k == KC - 1),
                        )
                        if k == KC - 1:
                            gst = sc * 4 + st
                            dst3 = v_sb[:, gst * VW:(gst + 1) * VW].rearrange(
                                "p (g x) -> p g x", x=E + 1)
                            nc.vector.tensor_copy(
                                dst3[:, :, 0:E],
                                ps[:, 0:DG].rearrange("p (g x) -> p g x", x=E))
                    return em

                for w_sb, is_q in ((wq_sb, True), (wk_sb, False)):
                    for g in range(2):
                        hold = {}
                        for k in range(KC):
                            ems.append(qk_step(k, w_sb, g, hold, is_q))
                for st in range(4):
                    hold = {}
                    for k in range(KC):
                        ems.append(v_step(k, st, hold))
                return ems

            # ---- dense prologue: project block 0 (own 3-buf psum scope) ----
            with tc.tile_pool(name="prop", bufs=3, space="PSUM") as prop:
                for em in proj_emitters(0, prop):
                    em()

            # ---- attention + injected projection / output projection ----
            # Flat software pipeline: ctx matmuls are emitted SKEW att-passes
            # after their scores matmul so PE never stalls on ACT's exp; the
            # normalize chain is staged in even later; out-proj for query
            # block ti and the projection of x block ti+1 are injected into
            # the attention stream of block ti.
            SK = 2
            INJ = {0: 5, 1: 3, 2: 2, 3: 0}   # proj emitters popped per pass
            with tc.tile_pool(name="prp", bufs=1, space="PSUM") as prp, \
                 tc.tile_pool(name="scp", bufs=3, space="PSUM") as scp, \
                 tc.tile_pool(name="cxp", bufs=2, space="PSUM") as cxp, \
                 tc.tile_pool(name="opp", bufs=2, space="PSUM") as opp, \
                 tc.tile_pool(name="pp_sb", bufs=6) as p_pool, \
                 tc.tile_pool(name="rb", bufs=2) as rbp, \
                 tc.tile_pool(name="ot", bufs=2) as otp:

                ctx_q = []    # (emit_fn, end_of_group_fn | None)
                due_q = []    # (passes_left, emit_fn) for staged normalize
                op_q = []     # pending out-proj emitters from previous block
                proj_q = []   # pending projection emitters for block ti+1
                norms_open = [0]  # groups whose cx write is not yet emitted
                inj_rate = [0]

                def emit_op(ti):
                    for it_ in range(4 * ti, 4 * ti + 4):
                        hold = {}
                        for dc in range(2):
                            def go(it_=it_, dc=dc, hold=hold):
                                ps = opp.tile([128, IT], F32, tag="ops")
                                for pair in range(2):
                                    nc.tensor.matmul(
                                        ps[:],
                                        lhsT=cx_sb[:, pair * S + it_ * JT: pair * S + it_ * JT + JT],
                                        rhs=wo_sb[:, pair * D + dc * IT: pair * D + (dc + 1) * IT],
                                        start=(pair == 0), stop=(pair == 1),
                                    )
                                ot = otp.tile([128, IT], BF16, name="ot", tag="ott")
                                nc.vector.tensor_copy(ot[:], ps[:])
                                eng = nc.sync
                                if it_ >= 12:
                                    eng = (nc.sync, nc.scalar, nc.gpsimd)[((it_ - 12) * 2 + dc) % 3]
                                eng.dma_start(
                                    out[it_ * JT:(it_ + 1) * JT, dc * IT:(dc + 1) * IT], ot[:])
                            op_q.append(go)

                norm_count = [0]

                def norm_stage_a(cps):
                    dn = dn_a if norm_count[0] % 2 == 0 else dn_b
                    norm_count[0] += 1
                    nc.vector.tensor_copy(dn[0:1, :], cps[E:E + 1, :])
                    return dn

                def norm_stage_b(cps, dn, h, ti):
                    qb, po = h // 2, 64 * (h % 2)
                    dnb = opp.tile([128, IT], F32, name="dnb", tag="ops")
                    nc.tensor.matmul(dnb[:], lhsT=ones_sb[:], rhs=dn[:],
                                     start=True, stop=True)
                    rcp = rbp.tile([128, IT], F32, tag="rcp")
                    nc.vector.reciprocal_approx_fast(rcp[0:E, :], dnb[0:E, :])
                    nc.vector.tensor_mul(
                        cx_sb[po:po + E, qb * S + ti * IT: qb * S + (ti + 1) * IT],
                        cps[0:E, :], rcp[0:E, :],
                    )
                    norms_open[0] -= 1

                def tick():
                    """Advance the pipeline by one att pass."""
                    for _ in range(inj_rate[0]):
                        if proj_q:
                            proj_q.pop(0)()
                    for e in list(due_q):
                        e[0] -= 1
                        if e[0] <= 0:
                            e[1]()
                            due_q.remove(e)
                    # out-proj reads cx, so it may only be emitted once the
                    # normalize stages that write cx have all been emitted
                    if op_q and not due_q and norms_open[0] == 0:
                        op_q.pop(0)()

                def drain_ctx():
                    emit, group_end = ctx_q.pop(0)
                    emit()
                    if group_end is not None:
                        group_end()

                for ti in range(NSC):
                    # block ti's projection must be fully emitted before its
                    # scores go on the PE queue (same-queue deadlock else)
                    while proj_q:
                        proj_q.pop(0)()
                    if ti + 1 < NSC:
                        proj_q.extend(proj_emitters(ti + 1, prp))
                    inj_rate[0] = INJ[ti]
                    njt = (IT // JT) * ti + (IT // JT)
                    for h in range(HG):
                        g = h // 2
                        cps = cxp.tile([128, IT], F32, tag="cps")
                        for jj in range(njt):
                            d = jj - (IT // JT) * ti
                            o = max(d, 0) * JT        # first valid query column
                            sp = scp.tile([128, IT], F32, tag="sp")
                            nc.tensor.matmul(
                                sp[:, o:IT],
                                lhsT=kT_sb[:, h * S + jj * JT: h * S + jj * JT + JT],
                                rhs=qT_sb[:, g * S + ti * IT + o: g * S + (ti + 1) * IT],
                                start=True, stop=True,
                            )
                            pt = p_pool.tile([128, IT], BF16, tag="pt")
                            nc.scalar.activation(pt[:, o:IT], sp[:, o:IT], EXP, scale=SCALE)
                            if d >= 0:
                                nc.gpsimd.tensor_mul(pt[:, o:o + JT], pt[:, o:o + JT], tri_sb[:])

                            def emit_ctx(cps=cps, pt=pt, h=h, jj=jj, o=o, njt=njt):
                                nc.tensor.matmul(
                                    cps[0:E + 1, o:IT],
                                    lhsT=v_sb[:, jj * VW + h * (E + 1): jj * VW + (h + 1) * (E + 1)],
                                    rhs=pt[:, o:IT],
                                    start=(jj == 0), stop=(jj == njt - 1),
                                )
                            group_end = None
                            if jj == njt - 1:
                                norms_open[0] += 1
                                def group_end(cps=cps, h=h, ti=ti):
                                    def stage_a(cps=cps, h=h, ti=ti):
                                        dn = norm_stage_a(cps)
                                        def stage_b(cps=cps, dn=dn, h=h, ti=ti):
                                            norm_stage_b(cps, dn, h, ti)
                                            due_q.append([2, lambda: None])
                                        due_q.append([4, stage_b])
                                    due_q.append([1, stage_a])
                            ctx_q.append((emit_ctx, group_end))
                            if len(ctx_q) > SK:
                                drain_ctx()
                            tick()
                    emit_op(ti)

                while ctx_q:
                    drain_ctx()
                    tick()
                for _ in range(60):
                    if not due_q and not op_q and not proj_q:
                        break
                    tick()
                assert not due_q and not op_q and not proj_q
                assert norms_open[0] == 0

    nc.compile()
    return nc


def _tri():
    # tri[jp, ic] = 1 where ic >= jp (keep), 0 above the causal boundary
    i = np.arange(JT)
    return (i[None, :] >= i[:, None]).astype(np.float32)


def _in_maps(x, Wq, Wk, Wv, Wo):
    tri = _tri()
    maps = []
    for c in range(NC):
        b, g = c // (NC // B), c % (NC // B)
        maps.append({
            "xT": np.ascontiguousarray(x[b].T).astype(ml_dtypes.bfloat16),
            "wq": np.ascontiguousarray(Wq[:, g * DG:(g + 1) * DG]).astype(ml_dtypes.bfloat16),
            "wk": np.ascontiguousarray(Wk[:, g * DG:(g + 1) * DG]).astype(ml_dtypes.bfloat16),
            "wv": np.ascontiguousarray(Wv[:, g * DG:(g + 1) * DG]).astype(ml_dtypes.bfloat16),
            "wo": np.ascontiguousarray(Wo[g * DG:(g + 1) * DG, :]),
            "tri": tri,
        })
    return maps


def run(x, Wq, Wk, Wv, Wo, bo, trace=False):
    if "nc" not in _CACHE:
        _CACHE["nc"] = _build()
    nc = _CACHE["nc"]
    res = bass_utils.run_bass_kernel_spmd(
        nc, _in_maps(x, Wq, Wk, Wv, Wo), core_ids=list(range(NC)), trace=trace,
    )
    parts = [np.asarray(res.results[c]["out"]).astype(np.float32) for c in range(NC)]
    gpb = NC // B
    full = np.stack([sum(parts[b * gpb + 1: (b + 1) * gpb], parts[b * gpb]) for b in range(B)])
    full = full + np.asarray(bo, np.float32)[None, None, :]
    return full.astype(np.float32), res


def kernel(x, Wq, Wk, Wv, Wo, bo):
    x = np.asarray(x, np.float32)
    full, _ = run(x, np.asarray(Wq, np.float32), np.asarray(Wk, np.float32),
                  np.asarray(Wv, np.float32), np.asarray(Wo, np.float32),
                  np.asarray(bo, np.float32))
    return full


# revision 21
# speedup vs baseline: 1.2523x; 1.0097x over previous
"""Multi-head causal attention (b=2, s=2048, d=1024, h=16) on 8 TRN2 cores.

Sharding: batch (2) x head-groups (4 heads each) -> 8 cores, Megatron-style.
Each core: QKV col-sliced projections (d -> 256), causal attention for its 4
heads, row-sliced output projection producing a partial [2048, 1024] output.
Host sums the 4 partials per batch and adds the output bias.

Device kernel layout:
  - x arrives pre-transposed (xT [1024, 2048]) and is streamed ONCE; q, k and
    v are all projected from the same resident [128, 8*512] x block per
    512-query chunk.
  - q, k are projected UNPADDED: head pair g = h//2 shares a psum tile
    ([128, 512]: head 2g on partitions 0-63, head 2g+1 on 64-127), halving
    the projection matmul count vs padded weights.
  - scores still contract over the FULL 128 partitions (half-array matmuls
    do not register as PE activity for the HAM clock gate, which then holds
    the whole attention phase at 1.2 GHz — measured): kT is stored per-head
    with the head's 64 rows at their natural partition range and the other
    64 rows zeroed once at startup; qT stays pair-packed and streams as the
    moving operand, its other-head rows multiplying the zero kT rows.
  - v natural ([s, head_dim]) with an extra ones column per head so the
    softmax denominator falls out of the ctx matmul (row 64 of ctx PSUM).
  - scores are computed transposed (p^T[j, i]) so the ctx matmul needs no
    transposes anywhere; softmax uses no max-subtraction (scores are O(5)
    for this distribution; exp is safe in fp32).
  - the Scalar (ACT) engine runs ONLY the exp stream — it is the attention
    bottleneck, so projection of x-block sc+1 is INJECTED into the attention
    pass stream of query block sc (like the out-proj injection) to overlap
    the whole projection phase with exp instead of serializing phases.
  - all matmuls run as float32r (fp22 multiply), full PE speed at
    moving >= 256 on TRN2.
"""

import ml_dtypes
import numpy as np

import concourse.bass as bass
import concourse.tile as tile
from concourse import bacc
from concourse import mybir
from concourse import bass_utils

F32 = mybir.dt.float32
F32R = mybir.dt.float32r
BF16 = mybir.dt.bfloat16
EXP = mybir.ActivationFunctionType.Exp

B, S, D, H = 2, 2048, 1024, 16
HG = 4                  # heads per core
E = 64                  # head dim
DG = HG * E             # 256, d-slice per core
NC = 8                  # cores
IT = 512                # query tile (moving dim of both attention matmuls)
JT = 128                # key tile
KC = D // 128           # 8 contraction chunks for projections
NSC = S // IT           # 4 s-chunks of 512
NST = S // JT           # 16 s-tiles of 128
SCALE = 1.0 / np.sqrt(E)

_CACHE = {}


def _build():
    nc = bacc.Bacc("TRN2", target_bir_lowering=False, debug=False)

    xT = nc.dram_tensor("xT", [D, S], BF16, kind="ExternalInput").ap()
    wq = nc.dram_tensor("wq", [D, DG], BF16, kind="ExternalInput").ap()
    wk = nc.dram_tensor("wk", [D, DG], BF16, kind="ExternalInput").ap()
    wv = nc.dram_tensor("wv", [D, DG], BF16, kind="ExternalInput").ap()
    wo = nc.dram_tensor("wo", [DG, D], F32R, kind="ExternalInput").ap()
    eye = nc.dram_tensor("eye", [JT, JT], BF16, kind="ExternalInput").ap()
    mneg = nc.dram_tensor("mneg", [JT, JT], BF16, kind="ExternalInput").ap()
    out = nc.dram_tensor("out", [S, D], BF16, kind="ExternalOutput").ap()

    with tile.TileContext(nc) as tc:
        from contextlib import ExitStack

        with ExitStack() as ctx:
            pers = ctx.enter_context(tc.tile_pool(name="pers", bufs=1))

            wq_sb = pers.tile([128, KC * DG], BF16, tag="wq")         # 4 KB/p
            wk_sb = pers.tile([128, KC * DG], BF16, tag="wk")
            wv_sb = pers.tile([128, KC * DG], BF16, tag="wv")
            wo_sb = pers.tile([128, 2 * D], F32R, tag="wo")           # 8 KB/p
            eye_sb = pers.tile([JT, JT], BF16, tag="eye")
            mneg_sb = pers.tile([JT, JT], BF16, tag="mneg")
            qT_sb = pers.tile([128, 2 * S], BF16, tag="qT")           # 8 KB/p
            kT_sb = pers.tile([128, HG * S], BF16, tag="kT")          # 16 KB/p
            v_sb = pers.tile([128, NST * (HG * (E + 1))], BF16, tag="v")  # 8.1 KB/p
            cx_sb = pers.tile([128, 2 * S], F32R, tag="cx")           # 16 KB/p
            ones_sb = pers.tile([128, 128], F32R, tag="ones")
            dn_a = pers.tile([128, IT], F32R, tag="dnpa")
            dn_b = pers.tile([128, IT], F32R, tag="dnpb")

            VW = HG * (E + 1)  # 260, v-row width per s-tile

            # ---- constants via engine memsets (a DMA'd ones-column scatter
            # measured 6+ us of descriptor grind). memset can't lower f32r
            # dsts — bitcast to plain f32 first. ----
            v3 = v_sb.rearrange("p (g x) -> p g x", x=E + 1)
            nc.vector.memset(ones_sb.bitcast(F32)[:], 1.0)
            nc.vector.memset(v3[:, :, E:E + 1], 1.0)
            nc.vector.memset(dn_a.bitcast(F32)[:], 0.0)
            nc.vector.memset(dn_b.bitcast(F32)[:], 0.0)
            # zero the dead 64-row half of each kT head block once; scores
            # matmuls then contract over the full 128 partitions
            for h in range(HG):
                po = 64 * (h % 2)
                dead = slice(64 - po, 128 - po)
                eng = nc.gpsimd if h % 2 == 0 else nc.vector
                eng.memset(kT_sb[dead, h * S:(h + 1) * S], 0.0)

            # ---- input loads. DMA issue is ~0.8 us apiece on an engine
            # queue, so loads are spread across idle engine queues (sync
            # keeps the critical wq/x chain, scalar takes wk/wo, gpsimd
            # takes wv behind its memsets), ordered so the first projection
            # matmul's chunks land first. ----
            wq3 = wq.rearrange("(k p) c -> p k c", p=128)
            wk3 = wk.rearrange("(k p) c -> p k c", p=128)
            wv3 = wv.rearrange("(k p) c -> p k c", p=128)
            wqs3 = wq_sb.rearrange("p (k c) -> p k c", c=DG)
            wks3 = wk_sb.rearrange("p (k c) -> p k c", c=DG)
            wvs3 = wv_sb.rearrange("p (k c) -> p k c", c=DG)
            h0, h1 = slice(0, 4), slice(4, 8)
            nc.sync.dma_start(eye_sb[:], eye[:])
            nc.sync.dma_start(mneg_sb[:], mneg[:])
            nc.sync.dma_start(wqs3[:, h0, :], wq3[:, h0, :])
            for hs in (h0, h1):
                nc.gpsimd.dma_start(wvs3[:, hs, :], wv3[:, hs, :])

            # warm the HAM clock gate while inputs stream: ~4.3 us of
            # dummy matmuls releases the 4/8 clock throttle right when the
            # first projection chunks land (result never read)
            with tc.tile_pool(name="wup", bufs=1, space="PSUM") as wup:
                wps = wup.tile([128, 128], F32, name="wps", tag="w")
                for i in range(40):
                    nc.tensor.matmul(wps[:], lhsT=ones_sb[:], rhs=ones_sb[:],
                                     start=(i == 0), stop=(i == 39))

            xv = xT.rearrange("(k p) s -> p k s", p=128)
            xp = ctx.enter_context(tc.tile_pool(name="xp", bufs=2))
            xts = []
            for sc in range(NSC):
                xt = xp.tile([128, KC * IT], BF16, tag="xt", name="xt")
                xt3 = xt.rearrange("p (k j) -> p k j", j=IT)
                if sc == 0:
                    nc.sync.dma_start(xt3[:, h0, :], xv[:, h0, 0:IT])
                    nc.scalar.dma_start(xt3[:, h1, :], xv[:, h1, 0:IT])
                    nc.scalar.dma_start(wqs3[:, h1, :], wq3[:, h1, :])
                    for hs in (h0, h1):
                        nc.scalar.dma_start(wks3[:, hs, :], wk3[:, hs, :])
                    for p in range(2):
                        nc.scalar.dma_start(
                            wo_sb[:, p * D:(p + 1) * D], wo[p * 128:(p + 1) * 128, :])
                else:
                    nc.sync.dma_start(xt3[:, :, :], xv[:, :, sc * IT:(sc + 1) * IT])
                xts.append(xt)

            # ---- projection emitters for one x block (q/k -> pair-packed
            # qT / zero-padded kT, v -> natural + ones col). Returned as a
            # flat list of single-matmul emitters so they can be injected
            # into the attention pass stream. ----
            def proj_emitters(sc, pool):
                ems = []
                xt = xts[sc]

                def qk_step(k, w_sb, g, hold, is_q):
                    def em():
                        if "ps" not in hold:
                            hold["ps"] = pool.tile([128, IT], F32, name="ps", tag="ps")
                        ps = hold["ps"]
                        nc.tensor.matmul(
                            ps[:],
                            lhsT=w_sb[:, k * DG + g * 128: k * DG + (g + 1) * 128],
                            rhs=xt[:, k * IT:(k + 1) * IT],
                            start=(k == 0), stop=(k == KC - 1),
                        )
                        if k == KC - 1:
                            if is_q:
                                nc.vector.tensor_copy(
                                    qT_sb[:, g * S + sc * IT: g * S + (sc + 1) * IT], ps[:])
                            else:
                                for hp in range(2):
                                    h = 2 * g + hp
                                    rows = slice(64 * hp, 64 * hp + 64)
                                    nc.vector.tensor_copy(
                                        kT_sb[rows, h * S + sc * IT: h * S + (sc + 1) * IT],
                                        ps[rows, :])
                    return em

                def v_step(k, st, hold):
                    def em():
                        if "ps" not in hold:
                            hold["ps"] = pool.tile([128, IT], F32, name="psv", tag="ps")
                        ps = hold["ps"]
                        nc.tensor.matmul(
                            ps[:, 0:DG],
                            lhsT=xt[:, k * IT + st * JT: k * IT + st * JT + JT],
                            rhs=wv_sb[:, k * DG:(k + 1) * DG],
                            start=(k == 0), stop=(k == KC - 1),
                        )
                        if k == KC - 1:
                            gst = sc * 4 + st
                            dst3 = v_sb[:, gst * VW:(gst + 1) * VW].rearrange(
                                "p (g x) -> p g x", x=E + 1)
                            nc.vector.tensor_copy(
                                dst3[:, :, 0:E],
                                ps[:, 0:DG].rearrange("p (g x) -> p g x", x=E))
                    return em

                for w_sb, is_q in ((wq_sb, True), (wk_sb, False)):
                    for g in range(2):
                        hold = {}
                        for k in range(KC):
                            ems.append(qk_step(k, w_sb, g, hold, is_q))
                for st in range(4):
                    hold = {}
                    for k in range(KC):
                        ems.append(v_step(k, st, hold))
                return ems

            # ---- dense prologue: project block 0 (own 3-buf psum scope) ----
            with tc.tile_pool(name="prop", bufs=3, space="PSUM") as prop:
                for em in proj_emitters(0, prop):
                    em()

            # ---- attention + injected projection / output projection ----
            # Flat software pipeline: ctx matmuls are emitted SKEW att-passes
            # after their scores matmul so PE never stalls on ACT's exp; the
            # normalize chain is staged in even later; out-proj for query
            # block ti and the projection of x block ti+1 are injected into
            # the attention stream of block ti.
            SK = 2
            INJ = {0: 5, 1: 3, 2: 2, 3: 0}   # proj emitters popped per pass
            with tc.tile_pool(name="prp", bufs=1, space="PSUM") as prp, \
                 tc.tile_pool(name="scp", bufs=3, space="PSUM") as scp, \
                 tc.tile_pool(name="cxp", bufs=2, space="PSUM") as cxp, \
                 tc.tile_pool(name="opp", bufs=2, space="PSUM") as opp, \
                 tc.tile_pool(name="pp_sb", bufs=6) as p_pool, \
                 tc.tile_pool(name="rb", bufs=2) as rbp, \
                 tc.tile_pool(name="ot", bufs=2) as otp:

                ctx_q = []    # (emit_fn, end_of_group_fn | None)
                due_q = []    # (passes_left, emit_fn) for staged normalize
                op_q = []     # pending out-proj emitters from previous block
                proj_q = []   # pending projection emitters for block ti+1
                norms_open = [0]  # groups whose cx write is not yet emitted
                inj_rate = [0]

                def emit_op(ti):
                    for it_ in range(4 * ti, 4 * ti + 4):
                        hold = {}
                        for dc in range(2):
                            def go(it_=it_, dc=dc, hold=hold):
                                ps = opp.tile([128, IT], F32, tag="ops")
                                for pair in range(2):
                                    nc.tensor.matmul(
                                        ps[:],
                                        lhsT=cx_sb[:, pair * S + it_ * JT: pair * S + it_ * JT + JT],
                                        rhs=wo_sb[:, pair * D + dc * IT: pair * D + (dc + 1) * IT],
                                        start=(pair == 0), stop=(pair == 1),
                                    )
                                ot = otp.tile([128, IT], BF16, name="ot", tag="ott")
                                nc.vector.tensor_copy(ot[:], ps[:])
                                eng = nc.sync
                                if it_ >= 12:
                                    eng = (nc.sync, nc.scalar, nc.gpsimd)[((it_ - 12) * 2 + dc) % 3]
                                eng.dma_start(
                                    out[it_ * JT:(it_ + 1) * JT, dc * IT:(dc + 1) * IT], ot[:])
                            op_q.append(go)

                norm_count = [0]

                def norm_stage_a(cps):
                    dn = dn_a if norm_count[0] % 2 == 0 else dn_b
                    norm_count[0] += 1
                    nc.vector.tensor_copy(dn[0:1, :], cps[E:E + 1, :])
                    return dn

                def norm_stage_b(cps, dn, h, ti):
                    qb, po = h // 2, 64 * (h % 2)
                    dnb = opp.tile([128, IT], F32, name="dnb", tag="ops")
                    nc.tensor.matmul(dnb[:], lhsT=ones_sb[:], rhs=dn[:],
                                     start=True, stop=True)
                    rcp = rbp.tile([128, IT], F32, tag="rcp")
                    nc.vector.reciprocal_approx_fast(rcp[0:E, :], dnb[0:E, :])
                    nc.vector.tensor_mul(
                        cx_sb[po:po + E, qb * S + ti * IT: qb * S + (ti + 1) * IT],
                        cps[0:E, :], rcp[0:E, :],
                    )
                    norms_open[0] -= 1

                def tick():
                    """Advance the pipeline by one att pass."""
                    for _ in range(inj_rate[0]):
                        if proj_q:
                            proj_q.pop(0)()
                    for e in list(due_q):
                        e[0] -= 1
                        if e[0] <= 0:
                            e[1]()
                            due_q.remove(e)
                    # out-proj reads cx, so it may only be emitted once the
                    # normalize stages that write cx have all been emitted
                    if op_q and not due_q and norms_open[0] == 0:
                        op_q.pop(0)()

                def drain_ctx():
                    emit, group_end = ctx_q.pop(0)
                    emit()
                    if group_end is not None:
                        group_end()

                for ti in range(NSC):
                    # block ti's projection must be fully emitted before its
                    # scores go on the PE queue (same-queue deadlock else)
                    while proj_q:
                        proj_q.pop(0)()
                    if ti + 1 < NSC:
                        proj_q.extend(proj_emitters(ti + 1, prp))
                    inj_rate[0] = INJ[ti]
                    njt = (IT // JT) * ti + (IT // JT)
                    for h in range(HG):
                        g = h // 2
                        cps = cxp.tile([128, IT], F32, tag="cps")
                        for jj in range(njt):
                            d = jj - (IT // JT) * ti
                            o = max(d, 0) * JT        # first valid query column
                            sp = scp.tile([128, IT], F32, tag="sp")
                            nc.tensor.matmul(
                                sp[:, o:IT],
                                lhsT=kT_sb[:, h * S + jj * JT: h * S + jj * JT + JT],
                                rhs=qT_sb[:, g * S + ti * IT + o: g * S + (ti + 1) * IT],
                                start=True, stop=(d < 0),
                            )
                            if d >= 0:
                                # fold the causal mask into the psum group:
                                # accumulate -1e30 above the diagonal so exp
                                # yields exact zeros — no engine hop between
                                # exp and the ctx matmul
                                nc.tensor.matmul(
                                    sp[:, o:o + JT], lhsT=eye_sb[:], rhs=mneg_sb[:],
                                    start=False, stop=True,
                                )
                            pt = p_pool.tile([128, IT], BF16, tag="pt")
                            nc.scalar.activation(pt[:, o:IT], sp[:, o:IT], EXP, scale=SCALE)

                            def emit_ctx(cps=cps, pt=pt, h=h, jj=jj, o=o, njt=njt):
                                nc.tensor.matmul(
                                    cps[0:E + 1, o:IT],
                                    lhsT=v_sb[:, jj * VW + h * (E + 1): jj * VW + (h + 1) * (E + 1)],
                                    rhs=pt[:, o:IT],
                                    start=(jj == 0), stop=(jj == njt - 1),
                                )
                            group_end = None
                            if jj == njt - 1:
                                norms_open[0] += 1
                                def group_end(cps=cps, h=h, ti=ti):
                                    def stage_a(cps=cps, h=h, ti=ti):
                                        dn = norm_stage_a(cps)
                                        def stage_b(cps=cps, dn=dn, h=h, ti=ti):
                                            norm_stage_b(cps, dn, h, ti)
                                            due_q.append([2, lambda: None])
                                        due_q.append([4, stage_b])
                                    due_q.append([1, stage_a])
                            ctx_q.append((emit_ctx, group_end))
                            if len(ctx_q) > SK:
                                drain_ctx()
                            tick()
                    emit_op(ti)

                while ctx_q:
                    drain_ctx()
                    tick()
                for _ in range(60):
                    if not due_q and not op_q and not proj_q:
                        break
                    tick()
                assert not due_q and not op_q and not proj_q
                assert norms_open[0] == 0

    nc.compile()
    return nc


def _tri():
    # tri[jp, ic] = 1 where ic >= jp (keep), 0 above the causal boundary
    i = np.arange(JT)
    return (i[None, :] >= i[:, None]).astype(np.float32)


def _in_maps(x, Wq, Wk, Wv, Wo):
    eye = np.eye(JT, dtype=np.float32).astype(ml_dtypes.bfloat16)
    mneg = np.where(_tri() > 0, 0.0, -1e30).astype(ml_dtypes.bfloat16)
    maps = []
    for c in range(NC):
        b, g = c // (NC // B), c % (NC // B)
        maps.append({
            "xT": np.ascontiguousarray(x[b].T).astype(ml_dtypes.bfloat16),
            "wq": np.ascontiguousarray(Wq[:, g * DG:(g + 1) * DG]).astype(ml_dtypes.bfloat16),
            "wk": np.ascontiguousarray(Wk[:, g * DG:(g + 1) * DG]).astype(ml_dtypes.bfloat16),
            "wv": np.ascontiguousarray(Wv[:, g * DG:(g + 1) * DG]).astype(ml_dtypes.bfloat16),
            "wo": np.ascontiguousarray(Wo[g * DG:(g + 1) * DG, :]),
            "eye": eye,
            "mneg": mneg,
        })
    return maps


def run(x, Wq, Wk, Wv, Wo, bo, trace=False):
    if "nc" not in _CACHE:
        _CACHE["nc"] = _build()
    nc = _CACHE["nc"]
    res = bass_utils.run_bass_kernel_spmd(
        nc, _in_maps(x, Wq, Wk, Wv, Wo), core_ids=list(range(NC)), trace=trace,
    )
    parts = [np.asarray(res.results[c]["out"]).astype(np.float32) for c in range(NC)]
    gpb = NC // B
    full = np.stack([sum(parts[b * gpb + 1: (b + 1) * gpb], parts[b * gpb]) for b in range(B)])
    full = full + np.asarray(bo, np.float32)[None, None, :]
    return full.astype(np.float32), res


def kernel(x, Wq, Wk, Wv, Wo, bo):
    x = np.asarray(x, np.float32)
    full, _ = run(x, np.asarray(Wq, np.float32), np.asarray(Wk, np.float32),
                  np.asarray(Wv, np.float32), np.asarray(Wo, np.float32),
                  np.asarray(bo, np.float32))
    return full
